# revision 1
# baseline (speedup 1.0000x reference)
"""Multi-head causal attention (b=2, s=2048, d=1024, h=16) on 8 TRN2 cores.

Sharding: batch (2) x head-groups (4 heads each) -> 8 cores, Megatron-style.
Each core: QKV col-sliced projections (d -> 256), causal attention for its 4
heads, row-sliced output projection producing a partial [2048, 1024] output.
Host sums the 4 partials per batch and adds the output bias.

Device kernel layout choices:
  - x arrives pre-transposed (xT [1024, 2048]) so all projections contract
    over the partition axis directly.
  - q, k are produced transposed ([head_dim, s], head_dim on partitions);
    v natural ([s, head_dim]) with an extra ones column per head so the
    softmax denominator falls out of the ctx matmul (row 64 of ctx PSUM).
  - scores are computed transposed (p^T[j, i]) so the ctx matmul needs no
    transposes anywhere; softmax uses no max-subtraction (scores are O(5)
    for this distribution; exp is safe in fp32).
  - all matmuls run as float32r (fp22 multiply) with moving dim >= 256,
    which is full PE speed on TRN2 at near-fp32 precision.
"""

import numpy as np

import concourse.bass as bass
import concourse.tile as tile
from concourse import bacc
from concourse import mybir
from concourse import bass_utils

F32 = mybir.dt.float32
F32R = mybir.dt.float32r
EXP = mybir.ActivationFunctionType.Exp
RECIP = mybir.ActivationFunctionType.Reciprocal

B, S, D, H = 2, 2048, 1024, 16
HG = 4                  # heads per core
E = 64                  # head dim
DG = HG * E             # 256, d-slice per core
NC = 8                  # cores
IT = 512                # query tile (moving dim of both attention matmuls)
JT = 128                # key tile
KC = D // 128           # 8 contraction chunks for projections
NSC = S // IT           # 4 s-chunks of 512
NST = S // JT           # 16 s-tiles of 128
SCALE = 1.0 / np.sqrt(E)

_CACHE = {}


def _build():
    nc = bacc.Bacc("TRN2", target_bir_lowering=False, debug=False)

    xT = nc.dram_tensor("xT", [D, S], F32R, kind="ExternalInput").ap()
    wq = nc.dram_tensor("wq", [D, 2 * DG], F32R, kind="ExternalInput").ap()
    wk = nc.dram_tensor("wk", [D, 2 * DG], F32R, kind="ExternalInput").ap()
    wv = nc.dram_tensor("wv", [D, DG], F32R, kind="ExternalInput").ap()
    wo = nc.dram_tensor("wo", [DG, D], F32R, kind="ExternalInput").ap()
    tri = nc.dram_tensor("tri", [JT, JT], F32, kind="ExternalInput").ap()
    one = nc.dram_tensor("one", [128, 128], F32R, kind="ExternalInput").ap()
    zed = nc.dram_tensor("zed", [128, IT], F32R, kind="ExternalInput").ap()
    out = nc.dram_tensor("out", [S, D], F32, kind="ExternalOutput").ap()

    with tile.TileContext(nc) as tc:
        from contextlib import ExitStack

        with ExitStack() as ctx:
            pers = ctx.enter_context(tc.tile_pool(name="pers", bufs=1))

            # persistent SBUF tensors (single tiles, sliced by AP)
            wq_sb = pers.tile([128, KC * 2 * DG], F32R, tag="wq")     # 16 KB/p
            wk_sb = pers.tile([128, KC * 2 * DG], F32R, tag="wk")
            wv_sb = pers.tile([128, KC * DG], F32R, tag="wv")
            wo_sb = pers.tile([128, 2 * D], F32R, tag="wo")           # 8 KB/p
            tri_sb = pers.tile([JT, JT], F32, tag="tri")
            qT_sb = pers.tile([128, HG * S], F32R, tag="qT")          # 32 KB/p
            kT_sb = pers.tile([128, HG * S], F32R, tag="kT")
            v_sb = pers.tile([128, NST * (HG * (E + 1))], F32R, tag="v")  # 16.25 KB/p
            cx_sb = pers.tile([128, 2 * S], F32R, tag="cx")           # 16 KB/p
            ones_sb = pers.tile([128, 128], F32R, tag="ones")
            dn_a = pers.tile([128, IT], F32R, tag="dnpa")
            dn_b = pers.tile([128, IT], F32R, tag="dnpb")

            VW = HG * (E + 1)  # 260, v-row width per s-tile

            # ---- loads (weights first so QK can start while xT streams) ----
            nc.sync.dma_start(tri_sb[:], tri[:])
            v3 = v_sb.rearrange("p (g x) -> p g x", x=E + 1)
            nc.sync.dma_start(v3[:, :, E:E + 1], one[:, 0:E, None])
            nc.sync.dma_start(ones_sb[:], one[:])
            nc.sync.dma_start(dn_a[:], zed[:])
            nc.sync.dma_start(dn_b[:], zed[:])
            WD = 2 * DG
            nc.sync.dma_start(wq_sb[:, 0:WD], wq[0:128, :])
            nc.sync.dma_start(wk_sb[:, 0:WD], wk[0:128, :])

            # ---- QKV projections ----
            # q/k per head with zero-padded weight columns: every matmul is a
            # full 128x128-mode op (half-array ops keep the PE HAM throttled
            # at K=4/8) and psum rows 64-127 come out zero, so padded qT/kT
            # copies are straight partition-aligned. xT is streamed in
            # [128, 512] slices per (sc, k) rather than kept resident.
            with tc.tile_pool(name="qkxt", bufs=4) as xtp, \
                 tc.tile_pool(name="qkvp", bufs=4, space="PSUM") as pp:
                for sc in range(NSC):
                    tiles = {}
                    for h in range(HG):
                        tiles[("q", h)] = pp.tile([128, IT], F32, name="psq", tag="psq")
                        tiles[("k", h)] = pp.tile([128, IT], F32, name="psk", tag="psk")
                    for k in range(KC):
                        xts = xtp.tile([128, IT], F32R, tag="xts")
                        nc.sync.dma_start(
                            xts[:], xT[k * 128:(k + 1) * 128, sc * IT:(sc + 1) * IT])
                        if sc == 0 and k + 1 < KC:
                            # just-in-time weight chunks so the first block's
                            # x stream isn't queued behind 4 MB of weights
                            kk = k + 1
                            nc.sync.dma_start(wq_sb[:, kk * WD:(kk + 1) * WD],
                                              wq[kk * 128:(kk + 1) * 128, :])
                            nc.sync.dma_start(wk_sb[:, kk * WD:(kk + 1) * WD],
                                              wk[kk * 128:(kk + 1) * 128, :])
                        for h in range(HG):
                            for w_sb, key in ((wq_sb, "q"), (wk_sb, "k")):
                                nc.tensor.matmul(
                                    tiles[(key, h)][:],
                                    lhsT=w_sb[:, k * 2 * DG + h * 128: k * 2 * DG + (h + 1) * 128],
                                    rhs=xts[:],
                                    start=(k == 0), stop=(k == KC - 1),
                                )
                    for h in range(HG):
                        dslice = slice(h * S + sc * IT, h * S + (sc + 1) * IT)
                        nc.scalar.copy(qT_sb[:, dslice], tiles[("q", h)][:])
                        nc.vector.tensor_copy(kT_sb[:, dslice], tiles[("k", h)][:])
            for k in range(KC):
                nc.sync.dma_start(wv_sb[:, k * DG:(k + 1) * DG], wv[k * 128:(k + 1) * 128, :])
            for p in range(2):
                nc.sync.dma_start(wo_sb[:, p * D:(p + 1) * D], wo[p * 128:(p + 1) * 128, :])
            with tc.tile_pool(name="vxt", bufs=3) as vxp, \
                 tc.tile_pool(name="qkvv", bufs=8, space="PSUM") as pv:
                SH = S // 2
                for vh in range(2):
                    vps = {}
                    for st in range(8):
                        vps[st] = pv.tile([128, DG], F32, name="psv", tag="psv")
                    for k in range(KC):
                        xtc = vxp.tile([128, SH], F32R, tag="xtc")
                        nc.sync.dma_start(
                            xtc[:], xT[k * 128:(k + 1) * 128, vh * SH:(vh + 1) * SH])
                        for st in range(8):
                            nc.tensor.matmul(
                                vps[st][:],
                                lhsT=xtc[:, st * JT:(st + 1) * JT],
                                rhs=wv_sb[:, k * DG:(k + 1) * DG],
                                start=(k == 0), stop=(k == KC - 1),
                            )
                    for st in range(8):
                        gst = vh * 8 + st
                        dst3 = v_sb[:, gst * VW:(gst + 1) * VW].rearrange("p (g x) -> p g x", x=E + 1)
                        nc.vector.tensor_copy(dst3[:, :, 0:E], vps[st].rearrange("p (g x) -> p g x", x=E))

            # ---- attention + interleaved output projection ----
            # Flat software pipeline: ctx matmuls are emitted SKEW att-passes
            # after their scores matmul so PE never stalls on ACT's exp; the
            # normalize chain is staged in even later; out-proj for query
            # block ti is injected into the attention stream of block ti+1.
            SK = 2
            with tc.tile_pool(name="scp", bufs=3, space="PSUM") as scp, \
                 tc.tile_pool(name="cxp", bufs=3, space="PSUM") as cxp, \
                 tc.tile_pool(name="opp", bufs=2, space="PSUM") as opp, \
                 tc.tile_pool(name="pp_sb", bufs=4) as p_pool, \
                 tc.tile_pool(name="rr", bufs=2) as rp, \
                 tc.tile_pool(name="rb", bufs=2) as rbp, \
                 tc.tile_pool(name="ot", bufs=2) as otp:

                ctx_q = []    # (emit_fn, end_of_group_fn | None)
                due_q = []    # (passes_left, emit_fn) for staged normalize
                op_q = []     # pending out-proj emitters from previous block
                norms_open = [0]  # groups whose cx write is not yet emitted

                def emit_op(ti):
                    for it_ in range(4 * ti, 4 * ti + 4):
                        for dc in range(2):
                            def go(it_=it_, dc=dc):
                                ps = opp.tile([128, IT], F32, tag="ops")
                                for pair in range(2):
                                    nc.tensor.matmul(
                                        ps[:],
                                        lhsT=cx_sb[:, pair * S + it_ * JT: pair * S + it_ * JT + JT],
                                        rhs=wo_sb[:, pair * D + dc * IT: pair * D + (dc + 1) * IT],
                                        start=(pair == 0), stop=(pair == 1),
                                    )
                                ot = otp.tile([128, IT], F32, tag="ott")
                                if dc == 0:
                                    nc.scalar.copy(ot[:], ps[:])
                                else:
                                    nc.vector.tensor_copy(ot[:], ps[:])
                                nc.sync.dma_start(
                                    out[it_ * JT:(it_ + 1) * JT, dc * IT:(dc + 1) * IT], ot[:])
                            op_q.append(go)

                norm_count = [0]

                def norm_stage_a(cps):
                    dn = dn_a if norm_count[0] % 2 == 0 else dn_b
                    norm_count[0] += 1
                    nc.vector.tensor_copy(dn[0:1, :], cps[E:E + 1, :])
                    return dn

                def norm_stage_b(cps, dn, h, ti):
                    qb, po = h // 2, 64 * (h % 2)
                    dnb = opp.tile([128, IT], F32, name="dnb", tag="ops")
                    nc.tensor.matmul(dnb[:], lhsT=ones_sb[:], rhs=dn[:],
                                     start=True, stop=True)
                    rcp = rbp.tile([128, IT], F32, tag="rcp")
                    nc.vector.reciprocal_approx_fast(rcp[0:E, :], dnb[0:E, :])
                    nc.vector.tensor_mul(
                        cx_sb[po:po + E, qb * S + ti * IT: qb * S + (ti + 1) * IT],
                        cps[0:E, :], rcp[0:E, :],
                    )
                    norms_open[0] -= 1

                def tick():
                    """Advance the pipeline by one att pass."""
                    for e in list(due_q):
                        e[0] -= 1
                        if e[0] <= 0:
                            e[1]()
                            due_q.remove(e)
                    # out-proj reads cx, so it may only be emitted once the
                    # normalize stages that write cx have all been emitted
                    if op_q and not due_q and norms_open[0] == 0:
                        op_q.pop(0)()

                def drain_ctx():
                    emit, group_end = ctx_q.pop(0)
                    emit()
                    if group_end is not None:
                        group_end()

                for ti in range(NSC):
                    njt = (IT // JT) * ti + (IT // JT)
                    for h in range(HG):
                        cps = cxp.tile([128, IT], F32, tag="cps")
                        for jj in range(njt):
                            d = jj - (IT // JT) * ti
                            o = max(d, 0) * JT        # first valid query column
                            sp = scp.tile([128, IT], F32, tag="sp")
                            nc.tensor.matmul(
                                sp[:, o:IT],
                                lhsT=kT_sb[:, h * S + jj * JT: h * S + jj * JT + JT],
                                rhs=qT_sb[:, h * S + ti * IT + o: h * S + (ti + 1) * IT],
                                start=True, stop=True,
                            )
                            pt = p_pool.tile([128, IT], F32R, tag="pt")
                            nc.scalar.activation(pt[:, o:IT], sp[:, o:IT], EXP, scale=SCALE)
                            if d >= 0:
                                nc.gpsimd.tensor_mul(pt[:, o:o + JT], pt[:, o:o + JT], tri_sb[:])

                            def emit_ctx(cps=cps, pt=pt, h=h, jj=jj, o=o, njt=njt):
                                nc.tensor.matmul(
                                    cps[0:E + 1, o:IT],
                                    lhsT=v_sb[:, jj * VW + h * (E + 1): jj * VW + (h + 1) * (E + 1)],
                                    rhs=pt[:, o:IT],
                                    start=(jj == 0), stop=(jj == njt - 1),
                                )
                            group_end = None
                            if jj == njt - 1:
                                norms_open[0] += 1
                                def group_end(cps=cps, h=h, ti=ti):
                                    def stage_a(cps=cps, h=h, ti=ti):
                                        dn = norm_stage_a(cps)
                                        def stage_b(cps=cps, dn=dn, h=h, ti=ti):
                                            norm_stage_b(cps, dn, h, ti)
                                            due_q.append([2, lambda: None])
                                        due_q.append([4, stage_b])
                                    due_q.append([1, stage_a])
                            ctx_q.append((emit_ctx, group_end))
                            if len(ctx_q) > SK:
                                drain_ctx()
                            tick()
                    emit_op(ti)

                while ctx_q:
                    drain_ctx()
                    tick()
                for _ in range(60):
                    if not due_q and not op_q:
                        break
                    tick()
                assert not due_q and not op_q and norms_open[0] == 0

    nc.compile()
    return nc


def _pad_heads(w):
    # [D, 256] -> [D, 512]: each 64-col head block padded to 128 with zeros
    wp = np.zeros((D, 2 * DG), np.float32)
    for h in range(HG):
        wp[:, h * 128: h * 128 + E] = w[:, h * E:(h + 1) * E]
    return wp


def _tri():
    # tri[jp, ic] = 1 where ic >= jp (keep), 0 above the causal boundary
    i = np.arange(JT)
    return (i[None, :] >= i[:, None]).astype(np.float32)


def _in_maps(x, Wq, Wk, Wv, Wo):
    tri = _tri()
    maps = []
    for c in range(NC):
        b, g = c // (NC // B), c % (NC // B)
        maps.append({
            "xT": np.ascontiguousarray(x[b].T),
            "wq": _pad_heads(Wq[:, g * DG:(g + 1) * DG]),
            "wk": _pad_heads(Wk[:, g * DG:(g + 1) * DG]),
            "wv": np.ascontiguousarray(Wv[:, g * DG:(g + 1) * DG]),
            "wo": np.ascontiguousarray(Wo[g * DG:(g + 1) * DG, :]),
            "tri": tri,
            "one": np.ones((128, 128), np.float32),
            "zed": np.zeros((128, IT), np.float32),
        })
    return maps


def run(x, Wq, Wk, Wv, Wo, bo, trace=False):
    if "nc" not in _CACHE:
        _CACHE["nc"] = _build()
    nc = _CACHE["nc"]
    res = bass_utils.run_bass_kernel_spmd(
        nc, _in_maps(x, Wq, Wk, Wv, Wo), core_ids=list(range(NC)), trace=trace,
    )
    parts = [res.results[c]["out"] for c in range(NC)]
    gpb = NC // B
    full = np.stack([sum(parts[b * gpb + 1: (b + 1) * gpb], parts[b * gpb]) for b in range(B)])
    full = full + np.asarray(bo, np.float32)[None, None, :]
    return full.astype(np.float32), res


def kernel(x, Wq, Wk, Wv, Wo, bo):
    x = np.asarray(x, np.float32)
    full, _ = run(x, np.asarray(Wq, np.float32), np.asarray(Wk, np.float32),
                  np.asarray(Wv, np.float32), np.asarray(Wo, np.float32),
                  np.asarray(bo, np.float32))
    return full



# revision 14
# speedup vs baseline: 1.2355x; 1.2355x over previous
"""Multi-head causal attention (b=2, s=2048, d=1024, h=16) on 8 TRN2 cores.

Sharding: batch (2) x head-groups (4 heads each) -> 8 cores, Megatron-style.
Each core: QKV col-sliced projections (d -> 256), causal attention for its 4
heads, row-sliced output projection producing a partial [2048, 1024] output.
Host sums the 4 partials per batch and adds the output bias.

v2 design (fused streaming pipeline):
  - One pass over s in 4 chunks of 512: projections for chunk sc are emitted
    interleaved with attention for query block sc-1, so PE, ACT (exp), DVE
    and GPSIMD stay busy end-to-end and the PE HAM clock never re-throttles.
  - Heads processed in PAIRS stacked on partition halves: q/k projections are
    unpadded [d,128] blocks (half the baseline's PE work), scores for the two
    heads of a pair run CONCURRENTLY as row-tiled K=64 matmuls (tile_position
    (0,0)/(64,0)), halving effective scores time.
  - The two concurrent score matmuls write one [128, 2, 512] PSUM supertile
    (2 banks) so a single ACT exp instruction covers both heads, halving the
    per-instruction ACT overhead (352 cycles/instr).
  - Causal mask via gpsimd.affine_select on the exp output (no tri tensor,
    no mask DMA); diagonal o-offsets clamped to <=256 so every fp32r matmul
    keeps moving dim >= 256 (below that fp32r runs 4x slower).
  - Softmax denominator from an extra ones-column in v (row 64 of ctx PSUM),
    broadcast via gpsimd.partition_broadcast (no PE broadcast matmul).
  - All inputs host-packed so every DMA is a contiguous per-partition blob:
    4 weight DMAs + 11 x DMAs + 32 output DMAs total.
"""

import numpy as np

import concourse.bass as bass
import concourse.tile as tile
from concourse import bacc
from concourse import mybir
from concourse import bass_utils

F32 = mybir.dt.float32
F32R = mybir.dt.float32r
EXP = mybir.ActivationFunctionType.Exp

B, S, D, H = 2, 2048, 1024, 16
HG = 4                  # heads per core
E = 64                  # head dim
DG = HG * E             # 256, d-slice per core
NC = 8                  # cores
IT = 512                # query block (moving dim of attention matmuls)
JT = 128                # key tile
KC = D // 128           # 8 contraction chunks for projections
NSC = S // IT           # 4 s-chunks of 512
SCALE = 1.0 / np.sqrt(E)
SK = 2                  # ctx-matmul skew (att pipeline depth, in units)

_CACHE = {}


def _build():
    nc = bacc.Bacc("TRN2", target_bir_lowering=False, debug=False)

    xp = nc.dram_tensor("xp", [128, NSC * KC * IT], F32R, kind="ExternalInput").ap()
    wq = nc.dram_tensor("wq", [128, KC * DG], F32R, kind="ExternalInput").ap()
    wk = nc.dram_tensor("wk", [128, KC * DG], F32R, kind="ExternalInput").ap()
    wv = nc.dram_tensor("wv", [128, KC * DG], F32R, kind="ExternalInput").ap()
    wo = nc.dram_tensor("wo", [128, 2 * D], F32R, kind="ExternalInput").ap()
    on = nc.dram_tensor("on", [128, 64], F32R, kind="ExternalInput").ap()
    out = nc.dram_tensor("out", [S, D], F32, kind="ExternalOutput").ap()

    with tile.TileContext(nc) as tc:
        from contextlib import ExitStack

        with ExitStack() as ctx:
            pers = ctx.enter_context(tc.tile_pool(name="pers", bufs=1))

            wq_sb = pers.tile([128, KC * DG], F32R, tag="wq")         # 8 KB/p
            wk_sb = pers.tile([128, KC * DG], F32R, tag="wk")
            wv_sb = pers.tile([128, KC * DG], F32R, tag="wv")
            wo_sb = pers.tile([128, 2 * D], F32R, tag="wo")
            qT_sb = pers.tile([128, 2 * S], F32R, tag="qT")           # 16 KB/p
            kT_sb = pers.tile([128, 2 * S], F32R, tag="kT")
            v_sb = pers.tile([128, 16 * HG * (E + 1)], F32R, tag="v")  # 16.25 KB/p
            cx_sb = pers.tile([128, 2 * S], F32R, tag="cx")

            v4 = v_sb.rearrange("p (t h x) -> p t h x", t=16, h=HG)
            VW = HG * (E + 1)  # 260

            xs_pool = ctx.enter_context(tc.tile_pool(name="xsp", bufs=3))
            pt_pool = ctx.enter_context(tc.tile_pool(name="ptp", bufs=3))
            ot_pool = ctx.enter_context(tc.tile_pool(name="otp", bufs=2))
            dn_pool = ctx.enter_context(tc.tile_pool(name="dnp", bufs=2))
            sp_pool = ctx.enter_context(tc.tile_pool(name="spp", bufs=2, space="PSUM"))
            cp_pool = ctx.enter_context(tc.tile_pool(name="cpp", bufs=2, space="PSUM"))
            pj_pool = ctx.enter_context(tc.tile_pool(name="pjp", bufs=1, space="PSUM"))
            op_pool = ctx.enter_context(tc.tile_pool(name="opp", bufs=1, space="PSUM"))

            zero_reg = nc.gpsimd.to_reg(0.0)

            # ---- prologue DMAs (all contiguous; first chunks split out so
            # the first matmul can start after ~2 small transfers) ----
            nc.sync.dma_start(wq_sb[:, 0:DG], wq[:, 0:DG])
            nc.sync.dma_start(wk_sb[:, 0:DG], wk[:, 0:DG])
            xs_tiles = []
            xs0 = xs_pool.tile([128, KC, IT], F32R, tag="xs", name="xs0")
            xs_tiles.append(xs0)
            for k in range(KC):
                nc.sync.dma_start(xs0[:, k, :], xp[:, k * IT:(k + 1) * IT])
            nc.sync.dma_start(wq_sb[:, DG:], wq[:, DG:])
            nc.sync.dma_start(wk_sb[:, DG:], wk[:, DG:])
            nc.sync.dma_start(wv_sb[:], wv[:])
            xs1 = xs_pool.tile([128, KC, IT], F32R, tag="xs", name="xs1")
            xs_tiles.append(xs1)
            nc.sync.dma_start(xs1[:], xp[:, KC * IT:2 * KC * IT])
            nc.sync.dma_start(wo_sb[:], wo[:])
            # ones column of v (softmax denominator rides the ctx matmul)
            ones_sb = pers.tile([128, 64], F32R, tag="ones")
            nc.sync.dma_start(ones_sb[:], on[:])
            nc.vector.tensor_copy(
                v4[:, :, :, E], ones_sb.rearrange("p (t h) -> p t h", h=HG))

            # ---- attention pipeline state machine ----
            ctx_q = []        # (emit_fn, group_end_fn | None)
            op_q = []         # pending out-proj emitters (prev query block)
            norms_open = [0]  # cx writes not yet emitted

            def tick():
                if op_q and norms_open[0] == 0:
                    op_q.pop(0)()

            def drain_ctx():
                emit, group_end = ctx_q.pop(0)
                emit()
                if group_end is not None:
                    group_end()

            cps_cur = {}      # head e -> cps tile for the pair in flight

            def emit_norm(cps, p, e, ti):
                # den row (part 64 of cps) -> broadcast -> recip -> scale ctx.
                # Emitted inline at group-end so the cps bank's next writer
                # (the following pair's first ctx matmul, drained one unit
                # later) is emitted after this read.
                dnr = dn_pool.tile([1, IT], F32, tag="dnr")
                nc.vector.tensor_copy(dnr[:, :], cps[E:E + 1, :])
                db = dn_pool.tile([64, IT], F32, tag="db")
                nc.gpsimd.partition_broadcast(db[:, :], dnr[:, :])
                rc = dn_pool.tile([64, IT], F32, tag="rc")
                nc.vector.reciprocal_approx_fast(rc[:, :], db[:, :])
                nc.vector.tensor_mul(
                    cx_sb[e * E:(e + 1) * E, p * S + ti * IT: p * S + (ti + 1) * IT],
                    cps[0:E, :], rc[:, :])
                norms_open[0] -= 1

            def att_unit(ti, p, jj, njt):
                def go():
                    d = jj - 4 * ti
                    o = min(max(d, 0) * JT, 256)
                    n = IT - o
                    sp = sp_pool.tile([128, 2, IT], F32, tag="sp")
                    for e in range(2):
                        # row-tiled K=64 pair: e=0 rows 0:64 / e=1 rows 64:128
                        # of the PE array run concurrently
                        nc.tensor.matmul(
                            sp[:, e, o:IT],
                            lhsT=kT_sb[e * E:(e + 1) * E, p * S + jj * JT: p * S + jj * JT + JT],
                            rhs=qT_sb[e * E:(e + 1) * E, p * S + ti * IT + o: p * S + (ti + 1) * IT],
                            start=True, stop=True,
                        )
                    pt = pt_pool.tile([128, 2, IT], F32R, tag="pt")
                    nc.scalar.activation(pt[:, :, o:IT], sp[:, :, o:IT], EXP, scale=SCALE)
                    if d >= 0:
                        # keep where query_pos >= key_pos
                        nc.gpsimd.affine_select(
                            pt[:, :, o:IT], pt[:, :, o:IT],
                            pattern=[[0, 2], [1, n]],
                            compare_op=mybir.AluOpType.is_ge,
                            fill=zero_reg,
                            base=ti * IT + o - jj * JT,
                            channel_multiplier=-1,
                        )

                    def emit_ctx(pt=pt, o=o, jj=jj, njt=njt):
                        if jj == 0:
                            for e in range(2):
                                cps_cur[e] = cp_pool.tile([128, IT], F32,
                                                          name="cps", tag="cps")
                        for e in range(2):
                            nc.tensor.matmul(
                                cps_cur[e][0:E + 1, o:IT],
                                lhsT=v_sb[:, jj * VW + (2 * p + e) * (E + 1):
                                          jj * VW + (2 * p + e + 1) * (E + 1)],
                                rhs=pt[:, e, o:IT],
                                start=(jj == 0), stop=(jj == njt - 1),
                            )
                    group_end = None
                    if jj == njt - 1:
                        norms_open[0] += 2
                        def group_end(p=p, ti=ti):
                            for e in range(2):
                                emit_norm(cps_cur[e], p, e, ti)
                    ctx_q.append((emit_ctx, group_end))
                    if len(ctx_q) > SK:
                        drain_ctx()
                    tick()
                return go

            def emit_op(ti):
                def go_all():
                    for it_ in range(4 * ti, 4 * ti + 4):
                        for dc in range(2):
                            def go(it_=it_, dc=dc):
                                ps = op_pool.tile([128, IT], F32, name="ops", tag="ops")
                                for pr in range(2):
                                    nc.tensor.matmul(
                                        ps[:],
                                        lhsT=cx_sb[:, pr * S + it_ * JT: pr * S + it_ * JT + JT],
                                        rhs=wo_sb[:, pr * D + dc * IT: pr * D + (dc + 1) * IT],
                                        start=(pr == 0), stop=(pr == 1),
                                    )
                                ot = ot_pool.tile([128, IT], F32, tag="ott")
                                nc.vector.tensor_copy(ot[:], ps[:])
                                nc.sync.dma_start(
                                    out[it_ * JT:(it_ + 1) * JT, dc * IT:(dc + 1) * IT],
                                    ot[:])
                            op_q.append(go)
                return go_all

            def att_units(ti):
                units = []
                njt = 4 * (ti + 1)
                for p in range(2):
                    for jj in range(njt):
                        units.append(att_unit(ti, p, jj, njt))
                units.append(emit_op(ti))
                return units

            # ---- projection emitters ----
            def qk_group(sc, pair, which):
                def go():
                    ps = pj_pool.tile([128, IT], F32, name="pjt", tag="pj")
                    w_sb = wq_sb if which == "q" else wk_sb
                    for k in range(KC):
                        nc.tensor.matmul(
                            ps[:],
                            lhsT=w_sb[:, k * DG + pair * 128: k * DG + (pair + 1) * 128],
                            rhs=xs_tiles[sc][:, k, :],
                            start=(k == 0), stop=(k == KC - 1),
                        )
                    dst = qT_sb if which == "q" else kT_sb
                    nc.vector.tensor_copy(
                        dst[:, pair * S + sc * IT: pair * S + (sc + 1) * IT], ps[:])
                return go

            def v_group(sc, st):
                def go():
                    ps = pj_pool.tile([128, IT], F32, name="pjt", tag="pj")
                    for k in range(KC):
                        nc.tensor.matmul(
                            ps[:, 0:DG],
                            lhsT=xs_tiles[sc][:, k, st * JT:(st + 1) * JT],
                            rhs=wv_sb[:, k * DG:(k + 1) * DG],
                            start=(k == 0), stop=(k == KC - 1),
                        )
                    nc.vector.tensor_copy(
                        v4[:, sc * 4 + st, :, 0:E],
                        ps[:, 0:DG].rearrange("p (h e) -> p h e", e=E))
                return go

            def proj_groups(sc):
                return [qk_group(sc, 0, "q"), qk_group(sc, 0, "k"),
                        qk_group(sc, 1, "q"), qk_group(sc, 1, "k"),
                        v_group(sc, 0), v_group(sc, 1),
                        v_group(sc, 2), v_group(sc, 3)]

            # ---- phase driver: weighted merge of proj groups + att units ----
            GCYC, UCYC = 4096, 1536
            for sc in range(NSC):
                if sc + 2 < NSC:
                    xs_n = xs_pool.tile([128, KC, IT], F32R, tag="xs",
                                        name=f"xs{sc + 2}")
                    xs_tiles.append(xs_n)
                    nc.sync.dma_start(
                        xs_n[:], xp[:, (sc + 2) * KC * IT:(sc + 3) * KC * IT])
                groups = proj_groups(sc)
                units = att_units(sc - 1) if sc >= 1 else []
                tp, tu = len(groups) * GCYC, len(units) * UCYC
                pc = uc = 0
                while groups or units:
                    if groups and (not units or pc * tu <= uc * tp):
                        groups.pop(0)()
                        pc += GCYC
                    else:
                        units.pop(0)()
                        uc += UCYC
            for u in att_units(NSC - 1):
                u()

            while ctx_q:
                drain_ctx()
                tick()
            for _ in range(80):
                if not op_q:
                    break
                tick()
            assert not op_q and norms_open[0] == 0

    nc.compile()
    return nc


def _pack_x(xb):
    # x[b] [2048, 1024] -> [128, 4*8*512]: chunk (sc, k) = xT[k*128:+128, sc*512:+512]
    return np.ascontiguousarray(
        xb.reshape(NSC, IT, KC, 128).transpose(3, 0, 2, 1).reshape(128, NSC * KC * IT))


def _pack_w(w):
    # [1024, 256] -> [128, 8*256] chunk-major
    return np.ascontiguousarray(
        w.reshape(KC, 128, DG).transpose(1, 0, 2).reshape(128, KC * DG))


def _pack_wo(w):
    # [256, 1024] -> [128, 2*1024] pair-major
    return np.ascontiguousarray(
        w.reshape(2, 128, D).transpose(1, 0, 2).reshape(128, 2 * D))


def _in_maps(x, Wq, Wk, Wv, Wo):
    maps = []
    for c in range(NC):
        b, g = c // (NC // B), c % (NC // B)
        maps.append({
            "xp": _pack_x(x[b]),
            "wq": _pack_w(Wq[:, g * DG:(g + 1) * DG]),
            "wk": _pack_w(Wk[:, g * DG:(g + 1) * DG]),
            "wv": _pack_w(Wv[:, g * DG:(g + 1) * DG]),
            "wo": _pack_wo(Wo[g * DG:(g + 1) * DG, :]),
            "on": np.ones((128, 64), np.float32),
        })
    return maps


def run(x, Wq, Wk, Wv, Wo, bo, trace=False):
    if "nc" not in _CACHE:
        _CACHE["nc"] = _build()
    nc = _CACHE["nc"]
    res = bass_utils.run_bass_kernel_spmd(
        nc, _in_maps(x, Wq, Wk, Wv, Wo), core_ids=list(range(NC)), trace=trace,
    )
    parts = [res.results[c]["out"] for c in range(NC)]
    gpb = NC // B
    full = np.stack([sum(parts[b * gpb + 1: (b + 1) * gpb], parts[b * gpb]) for b in range(B)])
    full = full + np.asarray(bo, np.float32)[None, None, :]
    return full.astype(np.float32), res


def kernel(x, Wq, Wk, Wv, Wo, bo):
    x = np.asarray(x, np.float32)
    full, _ = run(x, np.asarray(Wq, np.float32), np.asarray(Wk, np.float32),
                  np.asarray(Wv, np.float32), np.asarray(Wo, np.float32),
                  np.asarray(bo, np.float32))
    return full


# revision 16
# speedup vs baseline: 1.2957x; 1.0487x over previous
"""Multi-head causal attention (b=2, s=2048, d=1024, h=16) on 8 TRN2 cores.

Sharding: batch (2) x head-groups (4 heads each) -> 8 cores, Megatron-style.
Each core: QKV col-sliced projections (d -> 256), causal attention for its 4
heads, row-sliced output projection producing a partial [2048, 1024] output.
Host sums the 4 partials per batch and adds the output bias.

v2 design (fused streaming pipeline):
  - One pass over s in 4 chunks of 512: projections for chunk sc are emitted
    interleaved with attention for query block sc-1, so PE, ACT (exp), DVE
    and GPSIMD stay busy end-to-end and the PE HAM clock never re-throttles.
  - Heads processed in PAIRS stacked on partition halves: q/k projections are
    unpadded [d,128] blocks (half the baseline's PE work), scores for the two
    heads of a pair run CONCURRENTLY as row-tiled K=64 matmuls (tile_position
    (0,0)/(64,0)), halving effective scores time.
  - The two concurrent score matmuls write one [128, 2, 512] PSUM supertile
    (2 banks) so a single ACT exp instruction covers both heads, halving the
    per-instruction ACT overhead (352 cycles/instr).
  - Causal mask via gpsimd.affine_select on the exp output (no tri tensor,
    no mask DMA); diagonal o-offsets clamped to <=256 so every fp32r matmul
    keeps moving dim >= 256 (below that fp32r runs 4x slower).
  - Softmax denominator from an extra ones-column in v (row 64 of ctx PSUM),
    broadcast via gpsimd.partition_broadcast (no PE broadcast matmul).
  - All inputs host-packed so every DMA is a contiguous per-partition blob:
    4 weight DMAs + 11 x DMAs + 32 output DMAs total.
"""

import numpy as np

import concourse.bass as bass
import concourse.tile as tile
from concourse import bacc
from concourse import mybir
from concourse import bass_utils

F32 = mybir.dt.float32
F32R = mybir.dt.float32r
EXP = mybir.ActivationFunctionType.Exp

B, S, D, H = 2, 2048, 1024, 16
HG = 4                  # heads per core
E = 64                  # head dim
DG = HG * E             # 256, d-slice per core
NC = 8                  # cores
IT = 512                # query block (moving dim of attention matmuls)
JT = 128                # key tile
KC = D // 128           # 8 contraction chunks for projections
NSC = S // IT           # 4 s-chunks of 512
SCALE = 1.0 / np.sqrt(E)
SK = 2                  # ctx-matmul skew (att pipeline depth, in units)

_CACHE = {}


def _build():
    nc = bacc.Bacc("TRN2", target_bir_lowering=False, debug=False)

    xp = nc.dram_tensor("xp", [128, NSC * KC * IT], F32R, kind="ExternalInput").ap()
    wq = nc.dram_tensor("wq", [128, KC * DG], F32R, kind="ExternalInput").ap()
    wk = nc.dram_tensor("wk", [128, KC * DG], F32R, kind="ExternalInput").ap()
    wv = nc.dram_tensor("wv", [128, KC * DG], F32R, kind="ExternalInput").ap()
    wo = nc.dram_tensor("wo", [128, 2 * D], F32R, kind="ExternalInput").ap()
    on = nc.dram_tensor("on", [128, 64], F32R, kind="ExternalInput").ap()
    out = nc.dram_tensor("out", [S, D], F32, kind="ExternalOutput").ap()

    with tile.TileContext(nc) as tc:
        from contextlib import ExitStack

        with ExitStack() as ctx:
            pers = ctx.enter_context(tc.tile_pool(name="pers", bufs=1))

            wq_sb = pers.tile([128, KC * DG], F32R, tag="wq")         # 8 KB/p
            wk_sb = pers.tile([128, KC * DG], F32R, tag="wk")
            wv_sb = pers.tile([128, KC * DG], F32R, tag="wv")
            wo_sb = pers.tile([128, 2 * D], F32R, tag="wo")
            qT_sb = pers.tile([128, 2 * S], F32R, tag="qT")           # 16 KB/p
            kT_sb = pers.tile([128, 2 * S], F32R, tag="kT")
            v_sb = pers.tile([128, 16 * HG * (E + 1)], F32R, tag="v")  # 16.25 KB/p
            cx_sb = pers.tile([128, 2 * S], F32R, tag="cx")

            v4 = v_sb.rearrange("p (t h x) -> p t h x", t=16, h=HG)
            VW = HG * (E + 1)  # 260

            xs_pool = ctx.enter_context(tc.tile_pool(name="xsp", bufs=3))
            pt_pool = ctx.enter_context(tc.tile_pool(name="ptp", bufs=3))
            ot_pool = ctx.enter_context(tc.tile_pool(name="otp", bufs=2))
            dn_pool = ctx.enter_context(tc.tile_pool(name="dnp", bufs=2))
            sp_pool = ctx.enter_context(tc.tile_pool(name="spp", bufs=2, space="PSUM"))
            cp_pool = ctx.enter_context(tc.tile_pool(name="cpp", bufs=2, space="PSUM"))
            pj_pool = ctx.enter_context(tc.tile_pool(name="pjp", bufs=1, space="PSUM"))
            op_pool = ctx.enter_context(tc.tile_pool(name="opp", bufs=1, space="PSUM"))

            zero_reg = nc.gpsimd.to_reg(0.0)

            # ---- prologue DMAs (all contiguous; first chunks split out so
            # the first matmul can start after ~2 small transfers) ----
            nc.sync.dma_start(wq_sb[:, 0:DG], wq[:, 0:DG])
            nc.sync.dma_start(wk_sb[:, 0:DG], wk[:, 0:DG])
            xs_tiles = []
            xs0 = xs_pool.tile([128, KC, IT], F32R, tag="xs", name="xs0")
            xs_tiles.append(xs0)
            for k in range(KC):
                nc.sync.dma_start(xs0[:, k, :], xp[:, k * IT:(k + 1) * IT])
            nc.sync.dma_start(wq_sb[:, DG:], wq[:, DG:])
            nc.sync.dma_start(wk_sb[:, DG:], wk[:, DG:])
            nc.sync.dma_start(wv_sb[:], wv[:])
            xs1 = xs_pool.tile([128, KC, IT], F32R, tag="xs", name="xs1")
            xs_tiles.append(xs1)
            nc.sync.dma_start(xs1[:], xp[:, KC * IT:2 * KC * IT])
            nc.sync.dma_start(wo_sb[:], wo[:])
            # ones column of v (softmax denominator rides the ctx matmul)
            ones_sb = pers.tile([128, 64], F32R, tag="ones")
            nc.sync.dma_start(ones_sb[:], on[:])
            nc.vector.tensor_copy(
                v4[:, :, :, E], ones_sb.rearrange("p (t h) -> p t h", h=HG))

            # ---- attention pipeline state machine ----
            ctx_q = []        # (emit_fn, group_end_fn | None)
            op_q = []         # pending out-proj emitters (prev query block)
            norms_open = [0]  # cx writes not yet emitted

            def tick():
                if op_q and norms_open[0] == 0:
                    op_q.pop(0)()

            def drain_ctx():
                emit, group_end = ctx_q.pop(0)
                emit()
                if group_end is not None:
                    group_end()

            cps_cur = {}      # head e -> cps tile for the pair in flight

            def emit_norm(cps, p, e, ti):
                # den row (part 64 of cps) -> broadcast -> recip -> scale ctx.
                # Emitted inline at group-end so the cps bank's next writer
                # (the following pair's first ctx matmul, drained one unit
                # later) is emitted after this read.
                dnr = dn_pool.tile([1, IT], F32, tag="dnr")
                nc.vector.tensor_copy(dnr[:, :], cps[E:E + 1, :])
                db = dn_pool.tile([64, IT], F32, tag="db")
                nc.gpsimd.partition_broadcast(db[:, :], dnr[:, :])
                rc = dn_pool.tile([64, IT], F32, tag="rc")
                nc.vector.reciprocal_approx_fast(rc[:, :], db[:, :])
                nc.vector.tensor_mul(
                    cx_sb[e * E:(e + 1) * E, p * S + ti * IT: p * S + (ti + 1) * IT],
                    cps[0:E, :], rc[:, :])
                norms_open[0] -= 1

            def att_unit(ti, p, jj, njt):
                def go():
                    d = jj - 4 * ti
                    o = min(max(d, 0) * JT, 256)
                    n = IT - o
                    sp = sp_pool.tile([128, 2, IT], F32, tag="sp")
                    for e in range(2):
                        # row-tiled K=64 pair: e=0 rows 0:64 / e=1 rows 64:128
                        # of the PE array run concurrently
                        nc.tensor.matmul(
                            sp[:, e, o:IT],
                            lhsT=kT_sb[e * E:(e + 1) * E, p * S + jj * JT: p * S + jj * JT + JT],
                            rhs=qT_sb[e * E:(e + 1) * E, p * S + ti * IT + o: p * S + (ti + 1) * IT],
                            start=True, stop=True,
                        )
                    pt = pt_pool.tile([128, 2, IT], F32R, tag="pt")
                    nc.scalar.activation(pt[:, :, o:IT], sp[:, :, o:IT], EXP, scale=SCALE)
                    if d >= 0:
                        # keep where query_pos >= key_pos
                        nc.gpsimd.affine_select(
                            pt[:, :, o:IT], pt[:, :, o:IT],
                            pattern=[[0, 2], [1, n]],
                            compare_op=mybir.AluOpType.is_ge,
                            fill=zero_reg,
                            base=ti * IT + o - jj * JT,
                            channel_multiplier=-1,
                        )

                    def emit_ctx(pt=pt, o=o, jj=jj, njt=njt):
                        if jj == 0:
                            for e in range(2):
                                cps_cur[e] = cp_pool.tile([128, IT], F32,
                                                          name="cps", tag="cps")
                        for e in range(2):
                            nc.tensor.matmul(
                                cps_cur[e][0:E + 1, o:IT],
                                lhsT=v_sb[:, jj * VW + (2 * p + e) * (E + 1):
                                          jj * VW + (2 * p + e + 1) * (E + 1)],
                                rhs=pt[:, e, o:IT],
                                start=(jj == 0), stop=(jj == njt - 1),
                            )
                    group_end = None
                    if jj == njt - 1:
                        norms_open[0] += 2
                        def group_end(p=p, ti=ti):
                            for e in range(2):
                                emit_norm(cps_cur[e], p, e, ti)
                    ctx_q.append((emit_ctx, group_end))
                    if len(ctx_q) > SK:
                        drain_ctx()
                    tick()
                return go

            def emit_op(ti):
                # for the final block the proj pool is idle: alternate into it
                # so out-proj matmuls overlap the staging copies
                def go_all():
                    for it_ in range(4 * ti, 4 * ti + 4):
                        def go(it_=it_):
                            ot = ot_pool.tile([128, 2 * IT], F32, tag="ott")
                            for dc in range(2):
                                pool = pj_pool if (ti == NSC - 1 and dc == 1) else op_pool
                                tag = "pj" if (ti == NSC - 1 and dc == 1) else "ops"
                                ps = pool.tile([128, IT], F32, name="ops", tag=tag)
                                for pr in range(2):
                                    nc.tensor.matmul(
                                        ps[:],
                                        lhsT=cx_sb[:, pr * S + it_ * JT: pr * S + it_ * JT + JT],
                                        rhs=wo_sb[:, pr * D + dc * IT: pr * D + (dc + 1) * IT],
                                        start=(pr == 0), stop=(pr == 1),
                                    )
                                nc.vector.tensor_copy(ot[:, dc * IT:(dc + 1) * IT], ps[:])
                            nc.sync.dma_start(out[it_ * JT:(it_ + 1) * JT, :], ot[:])
                        op_q.append(go)
                return go_all

            # ---- projection emitters ----
            def qk_group(sc, pair, which):
                def go():
                    ps = pj_pool.tile([128, IT], F32, name="pjt", tag="pj")
                    w_sb = wq_sb if which == "q" else wk_sb
                    for k in range(KC):
                        nc.tensor.matmul(
                            ps[:],
                            lhsT=w_sb[:, k * DG + pair * 128: k * DG + (pair + 1) * 128],
                            rhs=xs_tiles[sc][:, k, :],
                            start=(k == 0), stop=(k == KC - 1),
                        )
                    dst = qT_sb if which == "q" else kT_sb
                    nc.vector.tensor_copy(
                        dst[:, pair * S + sc * IT: pair * S + (sc + 1) * IT], ps[:])
                return go

            def v_group(sc, st):
                def go():
                    ps = pj_pool.tile([128, IT], F32, name="pjt", tag="pj")
                    for k in range(KC):
                        nc.tensor.matmul(
                            ps[:, 0:DG],
                            lhsT=xs_tiles[sc][:, k, st * JT:(st + 1) * JT],
                            rhs=wv_sb[:, k * DG:(k + 1) * DG],
                            start=(k == 0), stop=(k == KC - 1),
                        )
                    nc.vector.tensor_copy(
                        v4[:, sc * 4 + st, :, 0:E],
                        ps[:, 0:DG].rearrange("p (h e) -> p h e", e=E))
                return go

            # ---- phase driver ----
            # Phase ti: projections for chunk ti AND attention for query
            # block ti (its off-diagonal units only need previous chunks'
            # k/v; diagonal units follow this chunk's k/v groups). Every
            # phase, including the last, has proj matmuls to fill the PE
            # while ACT chews on exp supertiles.
            def merge(groups, units, gcyc, ucyc):
                tp, tu = len(groups) * gcyc, len(units) * ucyc
                pc = uc = 0
                while groups or units:
                    if groups and (not units or pc * tu <= uc * tp):
                        groups.pop(0)()
                        pc += gcyc
                    else:
                        units.pop(0)()
                        uc += ucyc

            for ti in range(NSC):
                sc = ti
                if sc + 2 < NSC:
                    xs_n = xs_pool.tile([128, KC, IT], F32R, tag="xs",
                                        name=f"xs{sc + 2}")
                    xs_tiles.append(xs_n)
                    nc.sync.dma_start(
                        xs_n[:], xp[:, (sc + 2) * KC * IT:(sc + 3) * KC * IT])
                njt = 4 * (ti + 1)
                qk_group(sc, 0, "q")()
                qk_group(sc, 0, "k")()
                # pair0 off-diagonal units interleaved with remaining groups
                rest = [qk_group(sc, 1, "q"), qk_group(sc, 1, "k"),
                        v_group(sc, 0), v_group(sc, 1),
                        v_group(sc, 2), v_group(sc, 3)]
                p0_off = [att_unit(ti, 0, jj, njt) for jj in range(4 * ti)]
                merge(rest, p0_off, 3072, 1536)
                for jj in range(4 * ti, njt):          # pair0 diagonal
                    att_unit(ti, 0, jj, njt)()
                for jj in range(njt):                  # pair1
                    att_unit(ti, 1, jj, njt)()
                emit_op(ti)()

            while ctx_q:
                drain_ctx()
                tick()
            for _ in range(80):
                if not op_q:
                    break
                tick()
            assert not op_q and norms_open[0] == 0

    nc.compile()
    return nc


def _pack_x(xb):
    # x[b] [2048, 1024] -> [128, 4*8*512]: chunk (sc, k) = xT[k*128:+128, sc*512:+512]
    return np.ascontiguousarray(
        xb.reshape(NSC, IT, KC, 128).transpose(3, 0, 2, 1).reshape(128, NSC * KC * IT))


def _pack_w(w):
    # [1024, 256] -> [128, 8*256] chunk-major
    return np.ascontiguousarray(
        w.reshape(KC, 128, DG).transpose(1, 0, 2).reshape(128, KC * DG))


def _pack_wo(w):
    # [256, 1024] -> [128, 2*1024] pair-major
    return np.ascontiguousarray(
        w.reshape(2, 128, D).transpose(1, 0, 2).reshape(128, 2 * D))


def _in_maps(x, Wq, Wk, Wv, Wo):
    maps = []
    for c in range(NC):
        b, g = c // (NC // B), c % (NC // B)
        maps.append({
            "xp": _pack_x(x[b]),
            "wq": _pack_w(Wq[:, g * DG:(g + 1) * DG]),
            "wk": _pack_w(Wk[:, g * DG:(g + 1) * DG]),
            "wv": _pack_w(Wv[:, g * DG:(g + 1) * DG]),
            "wo": _pack_wo(Wo[g * DG:(g + 1) * DG, :]),
            "on": np.ones((128, 64), np.float32),
        })
    return maps


def run(x, Wq, Wk, Wv, Wo, bo, trace=False):
    if "nc" not in _CACHE:
        _CACHE["nc"] = _build()
    nc = _CACHE["nc"]
    res = bass_utils.run_bass_kernel_spmd(
        nc, _in_maps(x, Wq, Wk, Wv, Wo), core_ids=list(range(NC)), trace=trace,
    )
    parts = [res.results[c]["out"] for c in range(NC)]
    gpb = NC // B
    full = np.stack([sum(parts[b * gpb + 1: (b + 1) * gpb], parts[b * gpb]) for b in range(B)])
    full = full + np.asarray(bo, np.float32)[None, None, :]
    return full.astype(np.float32), res


def kernel(x, Wq, Wk, Wv, Wo, bo):
    x = np.asarray(x, np.float32)
    full, _ = run(x, np.asarray(Wq, np.float32), np.asarray(Wk, np.float32),
                  np.asarray(Wv, np.float32), np.asarray(Wo, np.float32),
                  np.asarray(bo, np.float32))
    return full
